# revision 16
# baseline (speedup 1.0000x reference)
"""Trainium2 Bass kernel for nn_AntisymmetricLayer.

Computes, per token n (with z = x1-x2, s = x1+x2):
    out[n,k] = sum_d z[n,d] W[k,d]  +  sum_r (sum_d z[n,d] P[k,d,r]) * (sum_d s[n,d] Q[k,d,r])

Sharding: data-parallel over the batch dim (8 batches -> 8 NeuronCores),
weights replicated, no collectives.

Per-core program (tokens N=16384, D=128, K=64, R=16):
  loop over 128-token tiles:
    PE   : transpose x1/x2 tiles with +/-identity accumulating into PSUM
           -> z^T, s^T (f32); then bf16 matmuls A = z @ P2 [128,1024],
           B = s @ Q2 [128,1024], lin = z @ W^T [128,64]
    ACT  : copy z^T/s^T PSUM -> SBUF with f32->bf16 cast
    DVE  : prod = A*B (PSUM x PSUM -> SBUF bf16), segmented reduce over
           r=16, add lin -> out tile [128,64] f32
    DMA  : chunked loads (512 tokens per dma), per-tile stores
"""

import numpy as np
import ml_dtypes

import concourse.bass as bass
import concourse.mybir as mybir
import concourse.tile as tile
from concourse import bacc
from concourse.bass import ts
from concourse.bass_utils import run_bass_kernel_spmd

F32 = mybir.dt.float32
BF16 = mybir.dt.bfloat16

D = 128
K = 64
R = 16
KR = K * R  # 1024
CONST_W = 2 * KR + K + 2 * 128  # p2|q2|wt|+I|-I packed
N_CORES = 8
TILE = 128          # tokens per tile (partition dim)
CHUNK_TILES = 4     # tiles loaded per input DMA (512 tokens)


def build_bass(n_tokens: int = 16384):
    """Build the per-core Bass program. All cores run the same program on
    their own batch shard."""
    assert n_tokens % (TILE * CHUNK_TILES) == 0
    n_tiles = n_tokens // TILE
    n_chunks = n_tiles // CHUNK_TILES

    nc = bacc.Bacc(None, target_bir_lowering=False)

    x1 = nc.declare_dram_parameter("x1", [n_tokens, D], F32, isOutput=False)
    x2 = nc.declare_dram_parameter("x2", [n_tokens, D], F32, isOutput=False)
    # all small constants packed into one tensor -> one load, one wait sem:
    # [p2 | q2 | wt | +I | -I] along the free dim
    cw = nc.declare_dram_parameter("cw", [D, CONST_W], BF16, isOutput=False)
    out = nc.declare_dram_parameter("out", [n_tokens, K], F32, isOutput=True)

    with tile.TileContext(nc) as tc:
        with (
            tc.tile_pool(name="const", bufs=1) as cpool,
            tc.tile_pool(name="xin", bufs=2) as xpool,
            tc.tile_pool(name="zs", bufs=3) as zpool,
            tc.tile_pool(name="prods", bufs=3) as ppool,
            tc.tile_pool(name="outs", bufs=3) as opool,
            tc.tile_pool(name="ptr", bufs=2, space="PSUM") as ptr_pool,
            tc.tile_pool(name="pab", bufs=2, space="PSUM") as pab_pool,
        ):
            # --- constants, loaded once (single DMA) ----------------------
            cws = cpool.tile([D, CONST_W], BF16)
            nc.sync.dma_start(cws[:], cw[:])
            p2s = cws[:, 0:KR]
            q2s = cws[:, KR : 2 * KR]
            wts = cws[:, 2 * KR : 2 * KR + K]
            ident = cws[:, 2 * KR + K : 2 * KR + K + D]
            identn = cws[:, 2 * KR + K + D : 2 * KR + K + 2 * D]

            x1v = x1.rearrange("(c a p) d -> c p a d", p=TILE, a=CHUNK_TILES)
            x2v = x2.rearrange("(c a p) d -> c p a d", p=TILE, a=CHUNK_TILES)

            # state carried across loop iterations for the 1-tile software
            # skew: tile i's transposes are emitted before tile i-1's matmuls
            prev = None  # (pz, ps, tile_idx)

            def do_tail(pz, ps, lin, i):
                # ACT: PSUM f32 -> SBUF bf16
                zt = zpool.tile([D, TILE], BF16, name=f"zt{i}", tag="zt")
                nc.scalar.copy(zt[:], pz[:])
                st = zpool.tile([D, TILE], BF16, name=f"st{i}", tag="st")
                nc.scalar.copy(st[:], ps[:])

                # PE: main matmuls (bf16, f32 accumulate)
                a0 = pab_pool.tile([TILE, 512], F32, name=f"a0_{i}", tag="A")
                a1 = pab_pool.tile([TILE, 512], F32, name=f"a1_{i}", tag="A")
                b0 = pab_pool.tile([TILE, 512], F32, name=f"b0_{i}", tag="B")
                b1 = pab_pool.tile([TILE, 512], F32, name=f"b1_{i}", tag="B")
                nc.tensor.matmul(a0[:], zt[:], p2s[:, 0:512], start=True, stop=True)
                nc.tensor.matmul(a1[:], zt[:], p2s[:, 512:1024], start=True, stop=True)
                nc.tensor.matmul(lin[:], zt[:], wts, start=True, stop=True)
                nc.tensor.matmul(b0[:], st[:], q2s[:, 0:512], start=True, stop=True)
                nc.tensor.matmul(b1[:], st[:], q2s[:, 512:1024], start=True, stop=True)

                # ACT: stage B in SBUF (bf16) -- DVE tensor_tensor may read at
                # most one PSUM operand
                b0s = ppool.tile([TILE, 512], BF16, name=f"b0s{i}", tag="b0s")
                nc.scalar.copy(b0s[:], b0[:])
                b1s = ppool.tile([TILE, 512], BF16, name=f"b1s{i}", tag="b1s")
                nc.scalar.copy(b1s[:], b1[:])

                # DVE: prod = A*B -> SBUF bf16 (one PSUM + one SBUF operand)
                prod = ppool.tile([TILE, KR], BF16, name=f"prod{i}", tag="prod")
                nc.vector.tensor_mul(prod[:, 0:512], a0[:], b0s[:])
                nc.vector.tensor_mul(prod[:, 512:1024], a1[:], b1s[:])

                # segmented reduce over r as a pairwise bf16 tree:
                # step 1 (largest) on GpSimd, rest on DVE (bf16 2x mode)
                pr = prod.rearrange("p (k r) -> p k r", r=R)
                t1 = opool.tile([TILE, K * 8], BF16, name=f"t1_{i}", tag="t1")
                t1v = t1.rearrange("p (k e) -> p k e", e=8)
                nc.gpsimd.tensor_add(t1v, pr[:, :, 0:8], pr[:, :, 8:16])
                t2 = opool.tile([TILE, K * 4], BF16, name=f"t2_{i}", tag="t2")
                t2v = t2.rearrange("p (k e) -> p k e", e=4)
                nc.vector.tensor_add(t2v, t1v[:, :, 0:4], t1v[:, :, 4:8])
                t3 = opool.tile([TILE, K * 2], BF16, name=f"t3_{i}", tag="t3")
                t3v = t3.rearrange("p (k e) -> p k e", e=2)
                nc.vector.tensor_add(t3v, t2v[:, :, 0:2], t2v[:, :, 2:4])
                red = opool.tile([TILE, K], F32, name=f"red{i}", tag="red")
                redv = red.rearrange("p (k o) -> p k o", o=1)
                nc.vector.tensor_add(redv, t3v[:, :, 0:1], t3v[:, :, 1:2])
                # DVE: add linear term
                fin = opool.tile([TILE, K], F32, name=f"fin{i}", tag="fin")
                nc.vector.tensor_add(fin[:], red[:], lin[:])
                nc.sync.dma_start(out[ts(i, TILE), :], fin[:])

            for c in range(n_chunks):
                # SWDGE DMA casts f32 DRAM -> bf16 SBUF in flight (full f32
                # read traffic from HBM, no compute-engine cost)
                x1c = xpool.tile([TILE, CHUNK_TILES, D], BF16, name=f"x1c{c}", tag="x1c")
                nc.gpsimd.dma_start(x1c[:], x1v[c])
                x2c = xpool.tile([TILE, CHUNK_TILES, D], BF16, name=f"x2c{c}", tag="x2c")
                nc.gpsimd.dma_start(x2c[:], x2v[c])

                for a in range(CHUNK_TILES):
                    i = c * CHUNK_TILES + a
                    # PE: transpose via regular matmul with +/-I as the
                    # moving operand: out = lhsT.T @ I = lhsT^T.
                    # combo spans 2 PSUM banks: bank0 holds pz (+ lin later),
                    # bank1 holds ps -- the two accumulation groups interleave
                    # but target different banks.
                    # pz = x1^T - x2^T ; ps = x1^T + x2^T   (f32, PSUM)
                    combo = ptr_pool.tile([D, 1024], F32, name=f"combo{i}", tag="trsp")
                    pz = combo[:, 0:TILE]
                    lin = combo[:, 256 : 256 + K]
                    ps = combo[:, 512 : 512 + TILE]
                    x1b = x1c[:, a, :]
                    x2b = x2c[:, a, :]
                    nc.tensor.matmul(pz, x1b, ident, start=True, stop=False)
                    nc.tensor.matmul(ps, x1b, ident, start=True, stop=False)
                    nc.tensor.matmul(pz, x2b, identn, start=False, stop=True)
                    nc.tensor.matmul(ps, x2b, ident, start=False, stop=True)

                    if prev is not None:
                        do_tail(*prev)
                    prev = (pz, ps, lin, i)

            do_tail(*prev)

    nc.finalize()
    return nc


def _shard_and_pack(x1, x2, W_lin, P, Q):
    """Host-side: batch-shard x1/x2, repack weights (layout + bf16 cast)."""
    p2 = P.transpose(1, 0, 2).reshape(D, KR)
    q2 = Q.transpose(1, 0, 2).reshape(D, KR)
    wt = np.ascontiguousarray(W_lin.T)
    idp = np.eye(D, dtype=np.float32)
    cw = np.concatenate([p2, q2, wt, idp, -idp], axis=1).astype(ml_dtypes.bfloat16)
    assert cw.shape == (D, CONST_W)

    in_maps = []
    for b in range(N_CORES):
        in_maps.append(
            {
                "x1": np.ascontiguousarray(x1[b]),
                "x2": np.ascontiguousarray(x2[b]),
                "cw": cw,
            }
        )
    return in_maps


def kernel(x1, x2, W_lin, P, Q):
    assert x1.shape == (N_CORES, 16384, D) and x2.shape == x1.shape
    nc = build_bass(16384)
    in_maps = _shard_and_pack(x1, x2, W_lin, P, Q)
    res = run_bass_kernel_spmd(nc, in_maps, core_ids=list(range(N_CORES)))
    out = np.stack([res.results[b]["out"] for b in range(N_CORES)], axis=0)
    return out.astype(np.float32)
